# revision 10
# baseline (speedup 1.0000x reference)
"""ConvGraphLayer kernel for 8 Trainium2 NeuronCores.

Computes: relu(concat([x, (adj @ x) / (nn + eps)], -1) @ fc_w.T + fc_b)

Strategy (1-D node/data parallel, per the row-shard hint):
  - Row-shard adj and num_neighbors across 8 cores (1250 rows each).
  - Each core's adjacency slice is staged host-side as contiguous columns of
    adj.T, so the contraction dim (source node k) lands on SBUF partitions and
    the tensor engine can consume it directly (no on-chip transpose needed).
  - x and fc weights are replicated; x_self / fc_w are staged pre-transposed
    (layout prep only) so every FLOP of the reference runs on-device.
  - Per core: nbT[f, i] = sum_k x[k, f] * adjT[k, i] accumulated in PSUM over
    79 k-tiles (float32r matmuls: fp32 data, full PE rate), scaled by
    1/(nn+eps), concatenated with x_selfT, then the fused FC+bias+relu.
"""

import sys

import numpy as np

try:
    import concourse.bacc as bacc
except ImportError:  # concourse ships in the container image, not on PyPI
    for _p in ("/opt/trn_rl_repo", "/root/.axon_site/_ro/trn_rl_repo"):
        if _p not in sys.path:
            sys.path.append(_p)
    import concourse.bacc as bacc

import concourse.mybir as mybir
import concourse.tile as tile
from concourse import bass_utils

N_NODES = 10000
F = 64
H = 64
EPS = 1e-7
N_CORES = 8
ROWS = N_NODES // N_CORES  # 1250 rows per core

F32 = mybir.dt.float32
F32R = mybir.dt.float32r

KT_FULL = N_NODES // 128          # 78 full k-tiles
K_REM = N_NODES - KT_FULL * 128   # 16 leftover contraction rows
# i-chunks: PSUM bank holds <=512 fp32; keep >=256 so float32r runs 1 cyc/row,
# and even widths (fp32r ISA: innermost free count of src/dst must be even)
ICHUNKS = [(0, 418), (418, 416), (834, 416)]

TRACE = False
TRACE_KWARGS = {}
LAST_RESULTS = None

_PROGRAM = None


def _build_body(tc, nc, adjT, x_full, x_selfT, nn_row, fc_wT, fc_b_col, out_rowsT):
    RELU = mybir.ActivationFunctionType.Relu
    COPY = mybir.ActivationFunctionType.Copy

    with (
        tc.tile_pool(name="const", bufs=1) as cpool,
        tc.tile_pool(name="stream", bufs=20) as spool,
        tc.tile_pool(name="psum", bufs=1, space="PSUM") as ppool,
    ):
        # x arrives host-pretiled in SBUF layout ([128, 79*64]): one clean DMA
        x_sb = cpool.tile([128, (KT_FULL + 1) * F], F32R, name="x_sb", tag="x_sb")
        nc.sync.dma_start(x_sb[:, :], x_full[:, :])

        # small constants + the k-tail load go out first (tiny; removes
        # end-of-kernel dependencies)
        adjt_tail = cpool.tile([K_REM, ROWS], F32R, name="adjt_tail", tag="adjt_tail")
        nc.sync.dma_start(adjt_tail[:, :], adjT[KT_FULL * 128 :, :])
        nn_sb = cpool.tile([1, ROWS], F32, name="nn_sb", tag="nn_sb")
        nc.sync.dma_start(nn_sb[:, :], nn_row[:, :])
        fcwT_sb = cpool.tile([2 * F, H], F32R, name="fcwT_sb", tag="fcwT_sb")
        nc.sync.dma_start(fcwT_sb[:, :], fc_wT[:, :])
        fcb_sb = cpool.tile([H, 1], F32, name="fcb_sb", tag="fcb_sb")
        nc.sync.dma_start(fcb_sb[:, :], fc_b_col[:, :])

        # 1/(nn + eps) + 64-partition broadcast, all before the k-loop so it
        # overlaps the adjacency stream instead of serializing after it
        nn_eps = cpool.tile([1, ROWS], F32, name="nn_eps", tag="nn_eps")
        nc.scalar.activation(nn_eps[:, :], nn_sb[:, :], COPY, bias=EPS)
        recip = cpool.tile([1, ROWS], F32, name="recip", tag="recip")
        nc.vector.reciprocal(recip[:, :], nn_eps[:, :])
        ones_f = cpool.tile([1, H], F32, name="ones_f", tag="ones_f")
        nc.vector.memset(ones_f[:, :], 1.0)
        rc_ps = [
            ppool.tile([128, w], F32, name=f"rc_ps{ci}", tag=f"rc_ps{ci}")
            for ci, (_, w) in enumerate(ICHUNKS)
        ]
        recip_sb = cpool.tile([128, ROWS], F32, name="recip_sb", tag="recip_sb")

        # catT rows 64..127 = x_selfT (rows 0..63 filled from nbT later)
        catT = cpool.tile([128, ROWS], F32R, name="catT", tag="catT")
        nc.sync.dma_start(catT[F : 2 * F, :], x_selfT[:, :])

        # nbT accumulators at PSUM partitions 0..63 (fp32r ISA: matmul dst
        # start_partition must be 0)
        nb_ps = [
            ppool.tile([128, w], F32, name=f"nb_ps{ci}", tag=f"nb_ps{ci}")
            for ci, (_, w) in enumerate(ICHUNKS)
        ]

        # main stream: one DMA + 3 accumulating matmuls per k-tile, so the PE
        # trails the DMA stream by at most one 640KB tile
        for kt in range(KT_FULL):
            adjt_sb = spool.tile([128, ROWS], F32R, name="adjt_sb", tag="adjt")
            nc.sync.dma_start(adjt_sb[:, :], adjT[kt * 128 : (kt + 1) * 128, :])
            lhs = x_sb[:, kt * F : (kt + 1) * F]
            for ci, (o, w) in enumerate(ICHUNKS):
                nc.tensor.matmul(
                    nb_ps[ci][0:64, :], lhs, adjt_sb[:, o : o + w],
                    start=(kt == 0), stop=(kt == KT_FULL - 1),
                )
            if kt == 1:
                # fold the K=16 tail into the stream early so it is not on the
                # end-of-kernel critical chain
                lhs_tail = x_sb[:K_REM, KT_FULL * F :]
                for ci, (o, w) in enumerate(ICHUNKS):
                    nc.tensor.matmul(
                        nb_ps[ci][0:64, :], lhs_tail, adjt_tail[:, o : o + w],
                        start=False, stop=False,
                    )
            if kt == 25:
                # reciprocal broadcast, long after its inputs are ready so the
                # in-order PE never stalls on it, long before the epilogue
                for ci, (o, w) in enumerate(ICHUNKS):
                    nc.tensor.matmul(
                        rc_ps[ci][0:64, :], ones_f[:, :], recip[:, o : o + w],
                        start=True, stop=True,
                    )
                    nc.scalar.activation(
                        recip_sb[0:64, o : o + w], rc_ps[ci][0:64, :], COPY
                    )

        # epilogue, chunk-pipelined: scale nbT -> catT, then one wide fp32r FC
        # matmul per chunk (weights stationary), relu+bias fused on ACT
        # reuse the rc_ps banks (already drained into recip_sb) for the FC
        oT_ps = [
            ppool.tile([128, w], F32, name=f"oT_ps{ci}", tag=f"rc_ps{ci}")
            for ci, (_, w) in enumerate(ICHUNKS)
        ]
        outT_sb = cpool.tile([H, ROWS], F32, name="outT_sb", tag="outT_sb")
        for ci, (o, w) in enumerate(ICHUNKS):
            nc.vector.tensor_mul(
                catT[0:64, o : o + w], nb_ps[ci][0:64, :], recip_sb[0:64, o : o + w]
            )
            nc.tensor.matmul(
                oT_ps[ci][0:64, :], fcwT_sb[:, :], catT[:, o : o + w],
                start=True, stop=True,
            )
            nc.scalar.activation(
                outT_sb[:, o : o + w], oT_ps[ci][0:64, :], RELU, bias=fcb_sb[:, :]
            )
            nc.sync.dma_start(out_rowsT[:, o : o + w], outT_sb[:, o : o + w])


def _get_program():
    global _PROGRAM
    if _PROGRAM is not None:
        return _PROGRAM
    nc = bacc.Bacc("TRN2", target_bir_lowering=False, debug=False)
    adjT = nc.dram_tensor("adjT_cols", [N_NODES, ROWS], F32R, kind="ExternalInput").ap()
    x_full = nc.dram_tensor("x_full", [128, (KT_FULL + 1) * F], F32R, kind="ExternalInput").ap()
    x_selfT = nc.dram_tensor("x_selfT", [F, ROWS], F32R, kind="ExternalInput").ap()
    nn_row = nc.dram_tensor("nn_row", [1, ROWS], F32, kind="ExternalInput").ap()
    fc_wT = nc.dram_tensor("fc_wT", [2 * F, H], F32R, kind="ExternalInput").ap()
    fc_b_col = nc.dram_tensor("fc_b_col", [H, 1], F32, kind="ExternalInput").ap()
    out_rowsT = nc.dram_tensor("out_rowsT", [H, ROWS], F32, kind="ExternalOutput").ap()

    with tile.TileContext(nc) as tc:
        _build_body(tc, nc, adjT, x_full, x_selfT, nn_row, fc_wT, fc_b_col, out_rowsT)
    nc.compile()
    _PROGRAM = nc
    return nc


def kernel(x, adj_matrix, num_neighbors, fc_w, fc_b):
    global LAST_RESULTS
    x = np.ascontiguousarray(np.asarray(x, dtype=np.float32))
    adj_matrix = np.asarray(adj_matrix, dtype=np.float32)
    num_neighbors = np.asarray(num_neighbors, dtype=np.float32)
    fc_w = np.asarray(fc_w, dtype=np.float32)
    fc_b = np.asarray(fc_b, dtype=np.float32)
    assert adj_matrix.shape == (N_NODES, N_NODES)

    # Host-side shard staging: adj.T column-blocks (= row shards of adj),
    # contiguous per core, in one pass.
    adjT_shards = np.ascontiguousarray(
        adj_matrix.T.reshape(N_NODES, N_CORES, ROWS).transpose(1, 0, 2)
    )
    xT = np.ascontiguousarray(x.T)  # [F, N]
    # x pre-tiled into the SBUF layout: tile t cols [t*F,(t+1)*F) = x rows t*128+p
    x_tiled = np.zeros((128, (KT_FULL + 1) * F), dtype=np.float32)
    x_tiled[:, : KT_FULL * F] = (
        x[: KT_FULL * 128].reshape(KT_FULL, 128, F).transpose(1, 0, 2).reshape(128, -1)
    )
    x_tiled[:K_REM, KT_FULL * F :] = x[KT_FULL * 128 :]

    in_maps = []
    for c in range(N_CORES):
        sl = slice(c * ROWS, (c + 1) * ROWS)
        in_maps.append(
            {
                "adjT_cols": adjT_shards[c],
                "x_full": x_tiled,
                "x_selfT": np.ascontiguousarray(xT[:, sl]),
                "nn_row": np.ascontiguousarray(num_neighbors[sl]).reshape(1, ROWS),
                "fc_wT": np.ascontiguousarray(
                    np.concatenate([fc_w[:, F:], fc_w[:, :F]], axis=1).T
                ),
                "fc_b_col": np.ascontiguousarray(fc_b).reshape(H, 1),
            }
        )

    nc = _get_program()
    results = bass_utils.run_bass_kernel_spmd(
        nc,
        in_maps,
        core_ids=list(range(N_CORES)),
        trace=TRACE,
        **TRACE_KWARGS,
    )
    LAST_RESULTS = results
    outs = [results.results[c]["out_rowsT"].T for c in range(N_CORES)]
    return np.ascontiguousarray(np.concatenate(outs, axis=0)).astype(
        np.float32, copy=False
    )


# revision 11
# speedup vs baseline: 1.1679x; 1.1679x over previous
"""ConvGraphLayer kernel for 8 Trainium2 NeuronCores.

Computes: relu(concat([x, (adj @ x) / (nn + eps)], -1) @ fc_w.T + fc_b)

Strategy (1-D node/data parallel, per the row-shard hint):
  - Row-shard adj and num_neighbors across 8 cores (1250 rows each).
  - Each core's adjacency slice is staged host-side as contiguous columns of
    adj.T, so the contraction dim (source node k) lands on SBUF partitions and
    the tensor engine can consume it directly (no on-chip transpose needed).
  - x and fc weights are replicated; x_self / fc_w are staged pre-transposed
    (layout prep only) so every FLOP of the reference runs on-device.
  - Per core: nbT[f, i] = sum_k x[k, f] * adjT[k, i] accumulated in PSUM over
    79 k-tiles (float32r matmuls: fp32 data, full PE rate), scaled by
    1/(nn+eps), concatenated with x_selfT, then the fused FC+bias+relu.
"""

import sys

import numpy as np

try:
    import concourse.bacc as bacc
except ImportError:  # concourse ships in the container image, not on PyPI
    for _p in ("/opt/trn_rl_repo", "/root/.axon_site/_ro/trn_rl_repo"):
        if _p not in sys.path:
            sys.path.append(_p)
    import concourse.bacc as bacc

import concourse.mybir as mybir
import concourse.tile as tile
from concourse import bass_utils

N_NODES = 10000
F = 64
H = 64
EPS = 1e-7
N_CORES = 8
ROWS = N_NODES // N_CORES  # 1250 rows per core

F32 = mybir.dt.float32
F32R = mybir.dt.float32r

KT_FULL = N_NODES // 128          # 78 full k-tiles
K_REM = N_NODES - KT_FULL * 128   # 16 leftover contraction rows
# i-chunks: PSUM bank holds <=512 fp32; keep >=256 so float32r runs 1 cyc/row,
# and even widths (fp32r ISA: innermost free count of src/dst must be even).
# The last chunk is deliberately narrow: it is the final stop->scale->FC->relu
# ->store chain after the DMA stream ends, so its width sets the kernel tail.
ICHUNKS = [(0, 512), (512, 482), (994, 256)]

TRACE = False
TRACE_KWARGS = {}
LAST_RESULTS = None

_PROGRAM = None


def _build_body(tc, nc, adjT, x_full, x_selfT, nn_row, fc_wT, fc_b_col, out_rowsT):
    RELU = mybir.ActivationFunctionType.Relu
    COPY = mybir.ActivationFunctionType.Copy

    with (
        tc.tile_pool(name="const", bufs=1) as cpool,
        tc.tile_pool(name="stream", bufs=20) as spool,
        tc.tile_pool(name="psum", bufs=1, space="PSUM") as ppool,
    ):
        # x arrives host-pretiled in SBUF layout ([128, 79*64]): one clean DMA
        x_sb = cpool.tile([128, (KT_FULL + 1) * F], F32R, name="x_sb", tag="x_sb")
        nc.sync.dma_start(x_sb[:, :], x_full[:, :])

        # small constants + the k-tail load go out first (tiny; removes
        # end-of-kernel dependencies)
        adjt_tail = cpool.tile([K_REM, ROWS], F32R, name="adjt_tail", tag="adjt_tail")
        nc.sync.dma_start(adjt_tail[:, :], adjT[KT_FULL * 128 :, :])
        nn_sb = cpool.tile([1, ROWS], F32, name="nn_sb", tag="nn_sb")
        nc.sync.dma_start(nn_sb[:, :], nn_row[:, :])
        fcwT_sb = cpool.tile([2 * F, H], F32R, name="fcwT_sb", tag="fcwT_sb")
        nc.sync.dma_start(fcwT_sb[:, :], fc_wT[:, :])
        fcb_sb = cpool.tile([H, 1], F32, name="fcb_sb", tag="fcb_sb")
        nc.sync.dma_start(fcb_sb[:, :], fc_b_col[:, :])

        # 1/(nn + eps) + 64-partition broadcast, all before the k-loop so it
        # overlaps the adjacency stream instead of serializing after it
        nn_eps = cpool.tile([1, ROWS], F32, name="nn_eps", tag="nn_eps")
        nc.scalar.activation(nn_eps[:, :], nn_sb[:, :], COPY, bias=EPS)
        recip = cpool.tile([1, ROWS], F32, name="recip", tag="recip")
        nc.vector.reciprocal(recip[:, :], nn_eps[:, :])
        ones_f = cpool.tile([1, H], F32, name="ones_f", tag="ones_f")
        nc.vector.memset(ones_f[:, :], 1.0)
        rc_ps = [
            ppool.tile([128, w], F32, name=f"rc_ps{ci}", tag=f"rc_ps{ci}")
            for ci, (_, w) in enumerate(ICHUNKS)
        ]
        recip_sb = cpool.tile([128, ROWS], F32, name="recip_sb", tag="recip_sb")

        # catT rows 64..127 = x_selfT (rows 0..63 filled from nbT later)
        catT = cpool.tile([128, ROWS], F32R, name="catT", tag="catT")
        nc.sync.dma_start(catT[F : 2 * F, :], x_selfT[:, :])

        # nbT accumulators at PSUM partitions 0..63 (fp32r ISA: matmul dst
        # start_partition must be 0)
        nb_ps = [
            ppool.tile([128, w], F32, name=f"nb_ps{ci}", tag=f"nb_ps{ci}")
            for ci, (_, w) in enumerate(ICHUNKS)
        ]

        # main stream: one DMA + 3 accumulating matmuls per k-tile, so the PE
        # trails the DMA stream by at most one 640KB tile
        for kt in range(KT_FULL):
            adjt_sb = spool.tile([128, ROWS], F32R, name="adjt_sb", tag="adjt")
            nc.sync.dma_start(adjt_sb[:, :], adjT[kt * 128 : (kt + 1) * 128, :])
            lhs = x_sb[:, kt * F : (kt + 1) * F]
            for ci, (o, w) in enumerate(ICHUNKS):
                nc.tensor.matmul(
                    nb_ps[ci][0:64, :], lhs, adjt_sb[:, o : o + w],
                    start=(kt == 0), stop=(kt == KT_FULL - 1),
                )
            if kt == 1:
                # fold the K=16 tail into the stream early so it is not on the
                # end-of-kernel critical chain
                lhs_tail = x_sb[:K_REM, KT_FULL * F :]
                for ci, (o, w) in enumerate(ICHUNKS):
                    nc.tensor.matmul(
                        nb_ps[ci][0:64, :], lhs_tail, adjt_tail[:, o : o + w],
                        start=False, stop=False,
                    )
            if kt == 25:
                # reciprocal broadcast, long after its inputs are ready so the
                # in-order PE never stalls on it, long before the epilogue
                for ci, (o, w) in enumerate(ICHUNKS):
                    nc.tensor.matmul(
                        rc_ps[ci][0:64, :], ones_f[:, :], recip[:, o : o + w],
                        start=True, stop=True,
                    )
                    nc.scalar.activation(
                        recip_sb[0:64, o : o + w], rc_ps[ci][0:64, :], COPY
                    )

        # epilogue, chunk-pipelined: scale nbT -> catT, then one wide fp32r FC
        # matmul per chunk (weights stationary), relu+bias fused on ACT
        # reuse the rc_ps banks (already drained into recip_sb) for the FC
        oT_ps = [
            ppool.tile([128, w], F32, name=f"oT_ps{ci}", tag=f"rc_ps{ci}")
            for ci, (_, w) in enumerate(ICHUNKS)
        ]
        outT_sb = cpool.tile([H, ROWS], F32, name="outT_sb", tag="outT_sb")
        for ci, (o, w) in enumerate(ICHUNKS):
            nc.vector.tensor_mul(
                catT[0:64, o : o + w], nb_ps[ci][0:64, :], recip_sb[0:64, o : o + w]
            )
            nc.tensor.matmul(
                oT_ps[ci][0:64, :], fcwT_sb[:, :], catT[:, o : o + w],
                start=True, stop=True,
            )
            nc.scalar.activation(
                outT_sb[:, o : o + w], oT_ps[ci][0:64, :], RELU, bias=fcb_sb[:, :]
            )
            nc.sync.dma_start(out_rowsT[:, o : o + w], outT_sb[:, o : o + w])


def _get_program():
    global _PROGRAM
    if _PROGRAM is not None:
        return _PROGRAM
    nc = bacc.Bacc("TRN2", target_bir_lowering=False, debug=False)
    adjT = nc.dram_tensor("adjT_cols", [N_NODES, ROWS], F32R, kind="ExternalInput").ap()
    x_full = nc.dram_tensor("x_full", [128, (KT_FULL + 1) * F], F32R, kind="ExternalInput").ap()
    x_selfT = nc.dram_tensor("x_selfT", [F, ROWS], F32R, kind="ExternalInput").ap()
    nn_row = nc.dram_tensor("nn_row", [1, ROWS], F32, kind="ExternalInput").ap()
    fc_wT = nc.dram_tensor("fc_wT", [2 * F, H], F32R, kind="ExternalInput").ap()
    fc_b_col = nc.dram_tensor("fc_b_col", [H, 1], F32, kind="ExternalInput").ap()
    out_rowsT = nc.dram_tensor("out_rowsT", [H, ROWS], F32, kind="ExternalOutput").ap()

    with tile.TileContext(nc) as tc:
        _build_body(tc, nc, adjT, x_full, x_selfT, nn_row, fc_wT, fc_b_col, out_rowsT)
    nc.compile()
    _PROGRAM = nc
    return nc


def kernel(x, adj_matrix, num_neighbors, fc_w, fc_b):
    global LAST_RESULTS
    x = np.ascontiguousarray(np.asarray(x, dtype=np.float32))
    adj_matrix = np.asarray(adj_matrix, dtype=np.float32)
    num_neighbors = np.asarray(num_neighbors, dtype=np.float32)
    fc_w = np.asarray(fc_w, dtype=np.float32)
    fc_b = np.asarray(fc_b, dtype=np.float32)
    assert adj_matrix.shape == (N_NODES, N_NODES)

    # Host-side shard staging: adj.T column-blocks (= row shards of adj),
    # contiguous per core, in one pass.
    adjT_shards = np.ascontiguousarray(
        adj_matrix.T.reshape(N_NODES, N_CORES, ROWS).transpose(1, 0, 2)
    )
    xT = np.ascontiguousarray(x.T)  # [F, N]
    # x pre-tiled into the SBUF layout: tile t cols [t*F,(t+1)*F) = x rows t*128+p
    x_tiled = np.zeros((128, (KT_FULL + 1) * F), dtype=np.float32)
    x_tiled[:, : KT_FULL * F] = (
        x[: KT_FULL * 128].reshape(KT_FULL, 128, F).transpose(1, 0, 2).reshape(128, -1)
    )
    x_tiled[:K_REM, KT_FULL * F :] = x[KT_FULL * 128 :]

    in_maps = []
    for c in range(N_CORES):
        sl = slice(c * ROWS, (c + 1) * ROWS)
        in_maps.append(
            {
                "adjT_cols": adjT_shards[c],
                "x_full": x_tiled,
                "x_selfT": np.ascontiguousarray(xT[:, sl]),
                "nn_row": np.ascontiguousarray(num_neighbors[sl]).reshape(1, ROWS),
                "fc_wT": np.ascontiguousarray(
                    np.concatenate([fc_w[:, F:], fc_w[:, :F]], axis=1).T
                ),
                "fc_b_col": np.ascontiguousarray(fc_b).reshape(H, 1),
            }
        )

    nc = _get_program()
    results = bass_utils.run_bass_kernel_spmd(
        nc,
        in_maps,
        core_ids=list(range(N_CORES)),
        trace=TRACE,
        **TRACE_KWARGS,
    )
    LAST_RESULTS = results
    outs = [results.results[c]["out_rowsT"].T for c in range(N_CORES)]
    return np.ascontiguousarray(np.concatenate(outs, axis=0)).astype(
        np.float32, copy=False
    )
